# revision 1
# baseline (speedup 1.0000x reference)
"""Trainium2 Bass kernel for nn_MoEConnectionProcessor.

Self-contained: stages/shards the full inputs on host (numpy), runs an SPMD
Bass/Tile kernel on 8 NeuronCores, gathers the full output.

Reference math (per cell, K=26 neighbors, D=32):
  masks by tier (0=local,1=functional,2=distant); masked neighbor means;
  local expert  = tanh([cs, loc_mean] @ W_local + b_local)
  func expert   = (1-z)*cs + z*tanh(agg),  z = sigmoid([cs, agg] @ W_upd + b_upd)
                  agg = masked_mean_k tanh(nb @ W_msg + b_msg)
  dist expert   = 3-step Euler: x += (1/3) tanh([x, agg_d] @ W_cnf + b_cnf)
  gates         = softmax([cs, mean_nb] @ W_g1 + b_g1 -> relu -> @ W_g2 + b_g2)
  out           = sum_t gate_t * expert_t

Device layout strategy per 128-cell tile (cells on SBUF partitions):
  - neighbor data staged natural [cells, (k d)] bf16; DVE StreamTranspose
    gives the d-on-partition operand for the PE matmul with a 4x block
    diagonal W_msg (contraction=32 features x 4 cell subgroups).
  - masked k-sums: DVE broadcast-AP multiplies + PE accumulation matmuls
    (constant identity stationary, 26 accumulating steps).
  - per-cell expert matmuls run in "block-T" layout (features on partitions,
    32-cell blocks) with 4x block-diagonal weights; biases become
    per-partition ACT bias vectors.
"""

import numpy as np
import ml_dtypes
from contextlib import ExitStack

import concourse.bass as bass
import concourse.bacc as bacc
import concourse.tile as tile
import concourse.mybir as mybir

B, K, D, NH = 262144, 26, 32, 32
N_CORES = 8
BS = B // N_CORES  # 32768 cells per core
CT = 128           # cells per tile
N_STEPS = 3
DT_STEP = 1.0 / N_STEPS

dt = mybir.dt
bf16 = ml_dtypes.bfloat16
AF = mybir.ActivationFunctionType
ALU = mybir.AluOpType

# column offsets into the packed weight-constant dram tensor [128, WC_COLS]
_WSLOTS = ["W4msg", "Wl_t", "Wl_b", "Wu_t", "Wu_b", "Wc_t", "Wc_b",
           "Wg1_t", "Wg1_b", "I128"]
WC_COLS = 128 * len(_WSLOTS) + 96  # + Wg2rep [128, 96]
BC_COLS = 8  # f32 bias consts


def _wslot(name):
    return 128 * _WSLOTS.index(name)


def build_program(bs=BS, ct=CT):
    """Builds the per-core Bass program (SPMD; all cores identical)."""
    nt = bs // ct
    nc = bacc.Bacc("TRN2", target_bir_lowering=False, debug=False,
                   num_devices=N_CORES)

    a_nbn = nc.dram_tensor("nbn", [bs, K * D], dt.bfloat16, kind="ExternalInput").ap()
    a_csn = nc.dram_tensor("csn", [bs, D], dt.float32, kind="ExternalInput").ap()
    a_cst = nc.dram_tensor("cst", [128, nt * D], dt.bfloat16, kind="ExternalInput").ap()
    a_msk = nc.dram_tensor("msk", [bs, 80], dt.bfloat16, kind="ExternalInput").ap()
    a_scl = nc.dram_tensor("scl", [bs, 4], dt.float32, kind="ExternalInput").ap()
    a_wc = nc.dram_tensor("wc", [128, WC_COLS], dt.bfloat16, kind="ExternalInput").ap()
    a_bc = nc.dram_tensor("bc", [128, BC_COLS], dt.float32, kind="ExternalInput").ap()
    a_out = nc.dram_tensor("out", [bs, D], dt.float32, kind="ExternalOutput").ap()

    with tile.TileContext(nc) as tc:
        _body(tc, a_nbn, a_csn, a_cst, a_msk, a_scl, a_wc, a_bc, a_out, bs, ct, nt)
    nc.compile()
    return nc


def _body(tc, a_nbn, a_csn, a_cst, a_msk, a_scl, a_wc, a_bc, a_out, bs, ct, nt):
    nc = tc.nc
    FR = K * D  # 832

    with ExitStack() as ctx:
        cpool = ctx.enter_context(tc.tile_pool(name="const", bufs=1))
        pin = ctx.enter_context(tc.tile_pool(name="in", bufs=3))
        psml = ctx.enter_context(tc.tile_pool(name="small", bufs=3))
        pbig = ctx.enter_context(tc.tile_pool(name="big", bufs=2))
        pps_m = ctx.enter_context(tc.tile_pool(name="psm", bufs=2, space="PSUM"))
        pps_s = ctx.enter_context(tc.tile_pool(name="pss", bufs=2, space="PSUM"))
        pps_d = ctx.enter_context(tc.tile_pool(name="psd", bufs=2, space="PSUM"))

        wc = cpool.tile([128, WC_COLS], dt.bfloat16, tag="wc")
        nc.sync.dma_start(wc[:], a_wc)
        bc = cpool.tile([128, BC_COLS], dt.float32, tag="bc")
        nc.sync.dma_start(bc[:], a_bc)

        def W(name):
            return wc[:, _wslot(name): _wslot(name) + 128]

        w2rep = wc[:, 128 * len(_WSLOTS): 128 * len(_WSLOTS) + 96]
        b_msg4 = bc[:, 0:1]
        b_loc4 = bc[:, 1:2]
        b_upd4 = bc[:, 2:3]
        b_cnf4 = bc[:, 3:4]
        b_g14 = bc[:, 4:5]
        bg2rep = bc[:, 5:8]  # [128, 3] replicated b_g2 row

        for t in range(nt):
            r0 = t * ct
            rows = slice(r0, r0 + ct)

            # ---- loads ----
            nb = pin.tile([128, FR], dt.bfloat16, tag="nb")
            nc.sync.dma_start(nb[:], a_nbn[rows, :])
            csn = psml.tile([128, D], dt.float32, tag="csn")
            nc.sync.dma_start(csn[:], a_csn[rows, :])
            cst = psml.tile([128, D], dt.bfloat16, tag="cst")
            nc.sync.dma_start(cst[:], a_cst[:, t * D:(t + 1) * D])
            msk = psml.tile([128, 80], dt.bfloat16, tag="msk")
            nc.sync.dma_start(msk[:], a_msk[rows, :])
            scl = psml.tile([128, 4], dt.float32, tag="scl")
            nc.sync.dma_start(scl[:], a_scl[rows, :])

            nb3 = nb[:].rearrange("p (k d) -> p k d", k=K)

            # ---- transpose for the message matmul ----
            nbT = pbig.tile([128, FR], dt.bfloat16, tag="nbT")
            nc.vector.transpose(nbT[:], nb[:])

            # ---- msgs = tanh(nb @ W_msg + b_msg), transposed layout ----
            ps_m0 = pps_m.tile([128, 416], dt.float32, tag="psm0")
            ps_m1 = pps_m.tile([128, 416], dt.float32, tag="psm1")
            nc.tensor.matmul(ps_m0[:], W("W4msg"), nbT[:, 0:416], start=True, stop=True)
            nc.tensor.matmul(ps_m1[:], W("W4msg"), nbT[:, 416:832], start=True, stop=True)
            msgsT = pbig.tile([128, FR], dt.bfloat16, tag="msgsT")
            nc.scalar.activation(msgsT[:, 0:416], ps_m0[:], AF.Tanh, bias=b_msg4, scale=1.0)
            nc.scalar.activation(msgsT[:, 416:832], ps_m1[:], AF.Tanh, bias=b_msg4, scale=1.0)

            # back to natural layout for the masked k-sum
            msgs_nat = pbig.tile([128, FR], dt.bfloat16, tag="msgsnat")
            nc.vector.transpose(msgs_nat[:], msgsT[:])
            msgs_nat3 = msgs_nat[:].rearrange("p (k d) -> p k d", k=K)

            # ---- masked products (broadcast-AP multiplies) ----
            def bmask(c0):
                return msk[:, c0:c0 + K].unsqueeze(2).to_broadcast((128, K, D))

            prodF = pbig.tile([128, FR], dt.bfloat16, tag="prodF")
            nc.vector.tensor_tensor(
                out=prodF[:].rearrange("p (k d) -> p k d", k=K),
                in0=msgs_nat3, in1=bmask(52), op=ALU.mult)
            prodA = pbig.tile([128, FR], dt.bfloat16, tag="prodA")
            nc.vector.tensor_tensor(
                out=prodA[:].rearrange("p (k d) -> p k d", k=K),
                in0=nb3, in1=bmask(0), op=ALU.mult)
            prodB = pbig.tile([128, FR], dt.bfloat16, tag="prodB")
            nc.vector.tensor_tensor(
                out=prodB[:].rearrange("p (k d) -> p k d", k=K),
                in0=nb3, in1=bmask(26), op=ALU.mult)

            # ---- k-sums via PE accumulation (identity stationary) ----
            # ps_sums columns: S0 @0, A @32, B @64, agg @96
            ps_sums = pps_s.tile([128, 128], dt.float32, tag="sums")
            srcs = [nb3, prodA[:].rearrange("p (k d) -> p k d", k=K),
                    prodB[:].rearrange("p (k d) -> p k d", k=K),
                    prodF[:].rearrange("p (k d) -> p k d", k=K)]
            for j, src in enumerate(srcs):
                for b in range(K):
                    nc.tensor.matmul(ps_sums[:, 32 * j:32 * j + 32], W("I128"),
                                     src[:, b, :], start=(b == 0), stop=(b == K - 1))

            S0 = ps_sums[:, 0:32]
            SA = ps_sums[:, 32:64]
            SB = ps_sums[:, 64:96]
            Sagg = ps_sums[:, 96:128]

            # ---- means (natural, f32) ----
            S0sb = psml.tile([128, D], dt.float32, tag="S0sb")
            nc.vector.tensor_copy(S0sb[:], S0)
            tmp_loc = psml.tile([128, D], dt.float32, tag="tmploc")
            nc.vector.tensor_tensor(out=tmp_loc[:], in0=S0sb[:], in1=SA, op=ALU.subtract)
            mean_loc = psml.tile([128, D], dt.bfloat16, tag="mloc")
            nc.vector.tensor_scalar(out=mean_loc[:], in0=tmp_loc[:],
                                    scalar1=scl[:, 0:1], scalar2=None, op0=ALU.mult)
            mean_dis = psml.tile([128, D], dt.bfloat16, tag="mdis")
            nc.vector.tensor_scalar(out=mean_dis[:], in0=SB,
                                    scalar1=scl[:, 1:2], scalar2=None, op0=ALU.mult)
            S0b16 = psml.tile([128, D], dt.bfloat16, tag="S0b16")
            nc.vector.tensor_copy(S0b16[:], S0sb[:])
            agg16 = psml.tile([128, D], dt.bfloat16, tag="agg16")
            nc.vector.tensor_copy(agg16[:], Sagg)

            # ---- tiny transposes into block-T layout (bf16 operands) ----
            mlT = psml.tile([128, D], dt.bfloat16, tag="mlT")
            nc.vector.transpose(mlT[:], mean_loc[:])
            mdT = psml.tile([128, D], dt.bfloat16, tag="mdT")
            nc.vector.transpose(mdT[:], mean_dis[:])
            mnT = psml.tile([128, D], dt.bfloat16, tag="mnT")
            nc.vector.transpose(mnT[:], S0b16[:])  # 1/K folded into Wg1_b on host
            aggT = psml.tile([128, D], dt.bfloat16, tag="aggT")
            nc.vector.transpose(aggT[:], agg16[:])
            xT = psml.tile([128, D], dt.float32, tag="xT")
            nc.vector.transpose(xT[:], csn[:])

            # ---- experts (block-T, PE + ACT) ----
            ps_dn = pps_d.tile([128, 192], dt.float32, tag="dn")

            nc.tensor.matmul(ps_dn[:, 0:32], W("Wl_t"), cst[:], start=True, stop=False)
            nc.tensor.matmul(ps_dn[:, 0:32], W("Wl_b"), mlT[:], start=False, stop=True)
            localT = psml.tile([128, D], dt.float32, tag="localT")
            nc.scalar.activation(localT[:], ps_dn[:, 0:32], AF.Tanh, bias=b_loc4, scale=1.0)

            nc.tensor.matmul(ps_dn[:, 32:64], W("Wu_t"), cst[:], start=True, stop=False)
            nc.tensor.matmul(ps_dn[:, 32:64], W("Wu_b"), aggT[:], start=False, stop=True)
            zT = psml.tile([128, D], dt.float32, tag="zT")
            nc.scalar.activation(zT[:], ps_dn[:, 32:64], AF.Sigmoid, bias=b_upd4, scale=1.0)

            nc.tensor.matmul(ps_dn[:, 64:96], W("Wg1_t"), cst[:], start=True, stop=False)
            nc.tensor.matmul(ps_dn[:, 64:96], W("Wg1_b"), mnT[:], start=False, stop=True)
            hT = psml.tile([128, D], dt.bfloat16, tag="hT")
            nc.scalar.activation(hT[:], ps_dn[:, 64:96], AF.Relu, bias=b_g14, scale=1.0)

            # CNF Euler steps (x kept f32, bf16 copies feed the PE)
            xcur = xT
            xbf = cst  # step-1 moving operand is exactly csT (bf16)
            for s in range(N_STEPS):
                nc.tensor.matmul(ps_dn[:, 128:160], W("Wc_t"), xbf[:], start=True, stop=False)
                nc.tensor.matmul(ps_dn[:, 128:160], W("Wc_b"), mdT[:], start=False, stop=True)
                vb = psml.tile([128, D], dt.float32, tag=f"vb{s}")
                nc.scalar.activation(vb[:], ps_dn[:, 128:160], AF.Tanh, bias=b_cnf4, scale=1.0)
                xnew = psml.tile([128, D], dt.float32, tag=f"xn{s}")
                nc.vector.scalar_tensor_tensor(out=xnew[:], in0=vb[:], scalar=DT_STEP,
                                               in1=xcur[:], op0=ALU.mult, op1=ALU.add)
                xcur = xnew
                if s < N_STEPS - 1:
                    xb2 = psml.tile([128, D], dt.bfloat16, tag=f"xb{s}")
                    nc.scalar.copy(xb2[:], xnew[:])
                    xbf = xb2

            # ---- gating (natural layout) ----
            h_nat = psml.tile([128, D], dt.bfloat16, tag="hnat")
            nc.vector.transpose(h_nat[:], hT[:])
            lg = psml.tile([128, 4], dt.float32, tag="lg")
            for g in range(3):
                gp = psml.tile([128, D], dt.bfloat16, tag="gp")
                nc.vector.tensor_tensor(out=gp[:], in0=h_nat[:],
                                        in1=w2rep[:, 32 * g:32 * g + 32], op=ALU.mult)
                nc.vector.tensor_reduce(out=lg[:, g:g + 1], in_=gp[:],
                                        axis=mybir.AxisListType.X, op=ALU.add)
            lgb = psml.tile([128, 3], dt.float32, tag="lgb")
            nc.vector.tensor_tensor(out=lgb[:], in0=lg[:, 0:3], in1=bg2rep, op=ALU.add)
            eg = psml.tile([128, 3], dt.float32, tag="eg")
            nc.scalar.activation(eg[:], lgb[:], AF.Exp)
            sg = psml.tile([128, 1], dt.float32, tag="sg")
            nc.vector.tensor_reduce(out=sg[:], in_=eg[:], axis=mybir.AxisListType.X, op=ALU.add)
            rinv = psml.tile([128, 1], dt.float32, tag="rinv")
            nc.vector.reciprocal(rinv[:], sg[:])
            gts = psml.tile([128, 3], dt.float32, tag="gts")
            nc.vector.tensor_scalar(out=gts[:], in0=eg[:], scalar1=rinv[:],
                                    scalar2=None, op0=ALU.mult)

            # ---- func expert combine (natural) ----
            tanh_agg = psml.tile([128, D], dt.float32, tag="tagg")
            nc.scalar.activation(tanh_agg[:], Sagg, AF.Tanh)
            z_nat = psml.tile([128, D], dt.float32, tag="znat")
            nc.vector.transpose(z_nat[:], zT[:])
            d2 = psml.tile([128, D], dt.float32, tag="d2")
            nc.vector.tensor_tensor(out=d2[:], in0=tanh_agg[:], in1=csn[:], op=ALU.subtract)
            f1 = psml.tile([128, D], dt.float32, tag="f1")
            nc.vector.tensor_tensor(out=f1[:], in0=z_nat[:], in1=d2[:], op=ALU.mult)
            func_nat = psml.tile([128, D], dt.float32, tag="func")
            nc.vector.tensor_tensor(out=func_nat[:], in0=f1[:], in1=csn[:], op=ALU.add)

            # ---- experts back to natural + weighted combine ----
            local_nat = psml.tile([128, D], dt.float32, tag="locnat")
            nc.vector.transpose(local_nat[:], localT[:])
            dist_nat = psml.tile([128, D], dt.float32, tag="distnat")
            nc.vector.transpose(dist_nat[:], xcur[:])

            acc1 = psml.tile([128, D], dt.float32, tag="acc1")
            nc.vector.tensor_scalar(out=acc1[:], in0=local_nat[:],
                                    scalar1=gts[:, 0:1], scalar2=None, op0=ALU.mult)
            acc2 = psml.tile([128, D], dt.float32, tag="acc2")
            nc.vector.scalar_tensor_tensor(out=acc2[:], in0=func_nat[:], scalar=gts[:, 1:2],
                                           in1=acc1[:], op0=ALU.mult, op1=ALU.add)
            acc3 = psml.tile([128, D], dt.float32, tag="acc3")
            nc.vector.scalar_tensor_tensor(out=acc3[:], in0=dist_nat[:], scalar=gts[:, 2:3],
                                           in1=acc2[:], op0=ALU.mult, op1=ALU.add)

            nc.sync.dma_start(a_out[rows, :], acc3[:])


# ---------------------------------------------------------------------------
# host staging
# ---------------------------------------------------------------------------

def stage_inputs(inputs, bs=BS, ct=CT):
    """Returns (in_maps, weights_dict) for run_bass_kernel_spmd."""
    nt = bs // ct
    cs = np.asarray(inputs["current_state"], np.float32)
    nb = np.asarray(inputs["neighbor_states"], np.float32)
    tiers = np.asarray(inputs["tier_ids"], np.int32)

    f32 = np.float32
    W_local = np.asarray(inputs["W_local"], f32)
    W_msg = np.asarray(inputs["W_msg"], f32)
    W_upd = np.asarray(inputs["W_upd"], f32)
    W_cnf = np.asarray(inputs["W_cnf"], f32)
    W_g1 = np.asarray(inputs["W_g1"], f32)
    W_g2 = np.asarray(inputs["W_g2"], f32)
    b_msg = np.asarray(inputs["b_msg"], f32)
    b_local = np.asarray(inputs["b_local"], f32)
    b_upd = np.asarray(inputs["b_upd"], f32)
    b_cnf = np.asarray(inputs["b_cnf"], f32)
    b_g1 = np.asarray(inputs["b_g1"], f32)
    b_g2 = np.asarray(inputs["b_g2"], f32)

    eye4 = np.eye(4, dtype=f32)

    def kron4(w):
        return np.kron(eye4, w)

    wparts = {
        "W4msg": kron4(W_msg),
        "Wl_t": kron4(W_local[:D]), "Wl_b": kron4(W_local[D:]),
        "Wu_t": kron4(W_upd[:D]), "Wu_b": kron4(W_upd[D:]),
        "Wc_t": kron4(W_cnf[:D]), "Wc_b": kron4(W_cnf[D:]),
        "Wg1_t": kron4(W_g1[:D]), "Wg1_b": kron4(W_g1[D:] / K),
        "I128": np.eye(128, dtype=f32),
    }
    wc = np.zeros((128, WC_COLS), f32)
    for name in _WSLOTS:
        wc[:, _wslot(name):_wslot(name) + 128] = wparts[name]
    for g in range(3):
        wc[:, 128 * len(_WSLOTS) + 32 * g: 128 * len(_WSLOTS) + 32 * g + 32] = W_g2[:, g][None, :]
    wc = wc.astype(bf16)

    bcq = np.zeros((128, BC_COLS), f32)
    bcq[:, 0] = np.tile(b_msg, 4)
    bcq[:, 1] = np.tile(b_local, 4)
    bcq[:, 2] = np.tile(b_upd, 4)
    bcq[:, 3] = np.tile(b_cnf, 4)
    bcq[:, 4] = np.tile(b_g1, 4)
    bcq[:, 5:8] = b_g2[None, :]

    in_maps = []
    for c in range(N_CORES):
        rs = slice(c * bs, (c + 1) * bs)
        nb_c = nb[rs]
        cs_c = cs[rs]
        tr_c = tiers[rs]

        nbn = nb_c.reshape(bs, K * D).astype(bf16)

        cs4 = cs_c.reshape(nt, 4, 32, D).transpose(0, 1, 3, 2)  # [t, a, d, c]
        cst = cs4.reshape(nt, 128, 32).transpose(1, 0, 2).reshape(128, nt * 32).astype(bf16)

        mA = (tr_c >= 1)
        mB = (tr_c == 2)
        m1 = (tr_c == 1)
        cnt0 = (tr_c == 0).sum(-1).astype(f32)
        cnt1 = m1.sum(-1).astype(f32)
        cnt2 = mB.sum(-1).astype(f32)
        wfun = m1.astype(f32) / np.maximum(cnt1, 1.0)[:, None]
        msk = np.zeros((bs, 80), f32)
        msk[:, 0:K] = mA
        msk[:, 26:26 + K] = mB
        msk[:, 52:52 + K] = wfun
        msk = msk.astype(bf16)

        scl = np.zeros((bs, 4), f32)
        scl[:, 0] = 1.0 / np.maximum(cnt0, 1.0)
        scl[:, 1] = 1.0 / np.maximum(cnt2, 1.0)

        in_maps.append({
            "nbn": nbn, "csn": cs_c.astype(f32), "cst": cst,
            "msk": msk, "scl": scl, "wc": wc, "bc": bcq,
        })
    return in_maps


_PROGRAM_CACHE = {}


def kernel(**inputs):
    from concourse.bass_utils import run_bass_kernel_spmd

    key = (BS, CT)
    if key not in _PROGRAM_CACHE:
        _PROGRAM_CACHE[key] = build_program(BS, CT)
    nc = _PROGRAM_CACHE[key]

    in_maps = stage_inputs(inputs, BS, CT)
    res = run_bass_kernel_spmd(nc, in_maps, core_ids=list(range(N_CORES)))
    out = np.concatenate([r["out"] for r in res.results], axis=0)
    return out.astype(np.float32)



# revision 6
# speedup vs baseline: 1.5583x; 1.5583x over previous
"""Trainium2 Bass kernel for nn_MoEConnectionProcessor.

Self-contained: stages/shards the full inputs on host (numpy), runs an SPMD
Bass/Tile kernel on 8 NeuronCores, gathers the full output.

Reference math (per cell, K=26 neighbors, D=32):
  masks by tier (0=local,1=functional,2=distant); masked neighbor means;
  local expert  = tanh([cs, loc_mean] @ W_local + b_local)
  func expert   = (1-z)*cs + z*tanh(agg),  z = sigmoid([cs, agg] @ W_upd + b_upd)
                  agg = masked_mean_k tanh(nb @ W_msg + b_msg)
  dist expert   = 3-step Euler: x += (1/3) tanh([x, agg_d] @ W_cnf + b_cnf)
  gates         = softmax([cs, mean_nb] @ W_g1 + b_g1 -> relu -> @ W_g2 + b_g2)
  out           = sum_t gate_t * expert_t

Device strategy (per 128-cell tile, Q=4 tiles batched for the small ops):
  - nb staged twice from host: T layout [(g,d), (c,k)] PRE-MASKED by the
    tier-1 mask (so tanh gives exact zeros for non-functional neighbors and
    the functional aggregate is a plain k-reduce), and natural [c, (d,k)]
    raw with k innermost (so the masked multiplies run in DVE 2x mode with
    the per-(cell,k) weights broadcast along d as an outer dim).
  - tier-0/tier-2 means: one fused 2x multiply by pre-divided weights
    (m_t/cnt_t, fp16) + one fused 1x k-reduce.
  - S0 (gating mean): PE identity-accumulation into PSUM (26 matmuls).
  - sigmoid via tanh identity, relu/exp/copy on ACT: every activation is
    served by the "exp_and_others" table -> zero ACT table reloads.
  - experts/gating/combine all in T layout on [128, 4*32] batched operands;
    per-cell gates/scales replicated across partitions with tiny PE matmuls;
    output staged in T layout, un-transposed on host.
"""

import numpy as np
import ml_dtypes
from contextlib import ExitStack

import concourse.bass as bass
import concourse.bacc as bacc
import concourse.tile as tile
import concourse.mybir as mybir

B, K, D, NH = 262144, 26, 32, 32
N_CORES = 8
BS = B // N_CORES   # 32768 cells per core
CT = 128            # cells per tile
QT = 4              # tiles per batch-group
N_STEPS = 3
DT_STEP = 1.0 / N_STEPS

dt = mybir.dt
bf16 = ml_dtypes.bfloat16
f16 = np.float16
AF = mybir.ActivationFunctionType
ALU = mybir.AluOpType
AXX = mybir.AxisListType

FR = K * D  # 832

# weight-constant dram tensor [128, WC_COLS] bf16 layout
_WSLOTS = ["W4msg", "Wl_t", "Wl_b", "Wu_t", "Wu_b", "Wc_t", "Wc_b",
           "Wg1_t", "Wg1_b", "I128", "REP4", "REPe0", "REPe1", "REPe2"]
_WEXTRA = 24  # WG2K [128,12] + SDEN [12,12 in a 12-col slot]
WC_COLS = 128 * len(_WSLOTS) + _WEXTRA
BC_COLS = 8


def _wslot(name):
    return 128 * _WSLOTS.index(name)


def build_program(bs=BS, ct=CT):
    nt = bs // ct
    nq = nt // QT
    nc = bacc.Bacc("TRN2", target_bir_lowering=False, debug=False,
                   num_devices=N_CORES)

    a_nbt = nc.dram_tensor("nbt", [128, nt * FR], dt.bfloat16, kind="ExternalInput").ap()
    a_nbn = nc.dram_tensor("nbn", [bs, FR], dt.bfloat16, kind="ExternalInput").ap()
    a_aux = nc.dram_tensor("aux", [bs, 2 * K], dt.float16, kind="ExternalInput").ap()
    a_cst = nc.dram_tensor("cst", [128, nt * D], dt.bfloat16, kind="ExternalInput").ap()
    a_scl = nc.dram_tensor("scl", [4, nt * D], dt.bfloat16, kind="ExternalInput").ap()
    a_wc = nc.dram_tensor("wc", [128, WC_COLS], dt.bfloat16, kind="ExternalInput").ap()
    a_bc = nc.dram_tensor("bc", [128, BC_COLS], dt.float32, kind="ExternalInput").ap()
    a_out = nc.dram_tensor("outt", [128, nt * D], dt.float32, kind="ExternalOutput").ap()

    with tile.TileContext(nc) as tc:
        _body(tc, a_nbt, a_nbn, a_aux, a_cst, a_scl, a_wc, a_bc, a_out,
              bs, ct, nt, nq)
    nc.compile()
    return nc


def _body(tc, a_nbt, a_nbn, a_aux, a_cst, a_scl, a_wc, a_bc, a_out,
          bs, ct, nt, nq):
    nc = tc.nc

    with ExitStack() as ctx:
        cpool = ctx.enter_context(tc.tile_pool(name="const", bufs=1))
        pin = ctx.enter_context(tc.tile_pool(name="in", bufs=3))
        pmid = ctx.enter_context(tc.tile_pool(name="mid", bufs=2))
        pq = ctx.enter_context(tc.tile_pool(name="q", bufs=2))
        psm = ctx.enter_context(tc.tile_pool(name="psm", bufs=2, space="PSUM"))
        psq = ctx.enter_context(tc.tile_pool(name="psq", bufs=2, space="PSUM"))
        psb = ctx.enter_context(tc.tile_pool(name="psb", bufs=1, space="PSUM"))

        wc = cpool.tile([128, WC_COLS], dt.bfloat16, tag="wc")
        nc.sync.dma_start(wc[:], a_wc)
        bc = cpool.tile([128, BC_COLS], dt.float32, tag="bc")
        nc.sync.dma_start(bc[:], a_bc)

        def W(name):
            return wc[:, _wslot(name): _wslot(name) + 128]

        wg2k = wc[:, 128 * len(_WSLOTS): 128 * len(_WSLOTS) + 12]
        sden = wc[0:12, 128 * len(_WSLOTS) + 12: 128 * len(_WSLOTS) + 24]
        b_msg4 = bc[:, 0:1]
        b_loc4 = bc[:, 1:2]
        hb_upd4 = bc[:, 2:3]   # 0.5 * b_upd (for the tanh-sigmoid identity)
        b_cnf4 = bc[:, 3:4]
        b_g14 = bc[:, 4:5]
        bg2c = bc[0:12, 5:6]   # b_g2 at (g,e) partitions

        for q in range(nq):
            # ---- per-Q staging tiles (filled by phase A) ----
            ML = pq.tile([128, QT * 2 * D], dt.float32, tag="ML")    # [c,(t,s,d)]
            SGT = pq.tile([128, QT * D], dt.float32, tag="SGT")      # [(g,j),(t,c)]
            ps_s0 = psq.tile([128, QT * D], dt.float32, tag="ps_s0")  # [c,(t,d)]

            # ================= phase A: per tile =================
            for s in range(QT):
                t = q * QT + s
                rows = slice(t * ct, (t + 1) * ct)

                nbt = pin.tile([128, FR], dt.bfloat16, tag="nbt")
                nc.gpsimd.dma_start(nbt[:], a_nbt[:, t * FR:(t + 1) * FR])
                nbn = pin.tile([128, FR], dt.bfloat16, tag="nbn")
                nc.gpsimd.dma_start(nbn[:], a_nbn[rows, :])
                aux = pin.tile([128, 2 * K], dt.float16, tag="aux")
                nc.gpsimd.dma_start(aux[:], a_aux[rows, :])

                nbn3 = nbn[:].rearrange("p (d k) -> p d k", d=D)

                # fused tier-0/tier-2 weighted products (DVE 2x)
                prod = pmid.tile([128, 2 * FR], dt.bfloat16, tag="prod")
                in0 = nbn[:].rearrange("p (d k) -> p d k", d=D) \
                    .unsqueeze(1).to_broadcast((128, 2, D, K))
                in1 = aux[:].rearrange("p (s k) -> p s k", s=2) \
                    .unsqueeze(2).to_broadcast((128, 2, D, K))
                nc.vector.tensor_tensor(
                    out=prod[:].rearrange("p (s d k) -> p s d k", s=2, d=D),
                    in0=in0, in1=in1, op=ALU.mult)

                # fused k-reduce -> [c, (s, d)] means (weights pre-divided)
                mlv = ML[:].rearrange("p (t s d) -> p t s d", t=QT, s=2)
                nc.vector.tensor_reduce(
                    out=mlv[:, s],
                    in_=prod[:].rearrange("p (s d k) -> p s d k", s=2, d=D),
                    axis=AXX.X, op=ALU.add)

                # messages: premsgs = kron4(W_msg) @ nbt  (nbt pre-masked m1)
                ps_m0 = psm.tile([128, 416], dt.float32, tag="psm0")
                ps_m1 = psm.tile([128, 416], dt.float32, tag="psm1")
                nc.tensor.matmul(ps_m0[:], W("W4msg"), nbt[:, 0:416],
                                 start=True, stop=True)
                nc.tensor.matmul(ps_m1[:], W("W4msg"), nbt[:, 416:832],
                                 start=True, stop=True)
                mt = pmid.tile([128, FR], dt.bfloat16, tag="msgsT")
                nc.scalar.activation(mt[:, 0:416], ps_m0[:], AF.Tanh,
                                     bias=b_msg4, scale=1.0)
                nc.scalar.activation(mt[:, 416:832], ps_m1[:], AF.Tanh,
                                     bias=b_msg4, scale=1.0)

                # functional aggregate: plain k-reduce (masked by staging)
                nc.vector.tensor_reduce(
                    out=SGT[:, s * D:(s + 1) * D],
                    in_=mt[:].rearrange("p (c k) -> p c k", k=K),
                    axis=AXX.X, op=ALU.add)

                # S0 = sum_k nb  (PE identity accumulation)
                for k in range(K):
                    nc.tensor.matmul(ps_s0[:, s * D:(s + 1) * D], W("I128"),
                                     nbn3[:, :, k],
                                     start=(k == 0), stop=(k == K - 1))

            # ================= phase B: per Q-group =================
            cstq = pin.tile([128, QT * D], dt.bfloat16, tag="cstq")
            nc.gpsimd.dma_start(cstq[:], a_cst[:, q * QT * D:(q + 1) * QT * D])
            sclq = pin.tile([128, QT * D], dt.bfloat16, tag="sclq")
            nc.gpsimd.dma_start(sclq[0:4, :], a_scl[:, q * QT * D:(q + 1) * QT * D])

            # T-transpose of the tier means (32x32 block transpose)
            MLTf = pq.tile([128, QT * 2 * D], dt.float32, tag="MLTf")
            nc.vector.transpose(MLTf[:], ML[:])
            MLT = pq.tile([128, QT * 2 * D], dt.bfloat16, tag="MLT")
            nc.vector.tensor_copy(MLT[:], MLTf[:])
            mltv = MLT[:].rearrange("p (t s c) -> p t s c", t=QT, s=2)
            mlT = mltv[:, :, 0]   # [(g,d), (t,c)] tier-0 means
            mdT = mltv[:, :, 1]   # tier-2 means

            # S0 -> bf16 -> T layout (gating mean; 1/K folded into Wg1_b)
            s0b = pq.tile([128, QT * D], dt.bfloat16, tag="s0b")
            nc.scalar.copy(s0b[:], ps_s0[:])
            mnT = pq.tile([128, QT * D], dt.bfloat16, tag="mnT")
            nc.vector.transpose(mnT[:], s0b[:])

            # two shared PSUM bank tiles for all phase-B matmul outputs
            PB1 = psb.tile([128, 512], dt.float32, tag="PB1")
            PB2 = psb.tile([128, 512], dt.float32, tag="PB2")
            ps_l = PB1[:, 0:128]
            ps_z = PB1[:, 128:256]
            ps_h = PB1[:, 256:384]
            ps_c = PB1[:, 384:512]
            ps_w = PB2[:, 0:384]
            ps_x = PB2[:, 384:512]   # reused: scl-rep, gating logits, denom

            # replicate 1/cnt1 across partitions; scale + tanh the aggregate
            ps_scl = ps_x
            nc.tensor.matmul(ps_scl, W("REP4")[0:4, :], sclq[0:4, :],
                             start=True, stop=True)
            aggTs = pq.tile([128, QT * D], dt.bfloat16, tag="aggTs")
            nc.vector.tensor_tensor(out=aggTs[:], in0=SGT[:], in1=ps_scl,
                                    op=ALU.mult)
            taggT = pq.tile([128, QT * D], dt.bfloat16, tag="taggT")
            nc.scalar.activation(taggT[:], aggTs[:], AF.Tanh)

            # ---- experts (batched matmuls over [128, QT*32]) ----
            nc.tensor.matmul(ps_l, W("Wl_t"), cstq[:], start=True, stop=False)
            nc.tensor.matmul(ps_l, W("Wl_b"), mlT, start=False, stop=True)
            localT = pq.tile([128, QT * D], dt.bfloat16, tag="localT")
            nc.scalar.activation(localT[:], ps_l, AF.Tanh, bias=b_loc4, scale=1.0)

            nc.tensor.matmul(ps_z, W("Wu_t"), cstq[:], start=True, stop=False)
            nc.tensor.matmul(ps_z, W("Wu_b"), aggTs[:], start=False, stop=True)
            tauT = pq.tile([128, QT * D], dt.bfloat16, tag="tauT")
            nc.scalar.activation(tauT[:], ps_z, AF.Tanh, bias=hb_upd4, scale=0.5)

            nc.tensor.matmul(ps_h, W("Wg1_t"), cstq[:], start=True, stop=False)
            nc.tensor.matmul(ps_h, W("Wg1_b"), mnT[:], start=False, stop=True)
            hT = pq.tile([128, QT * D], dt.bfloat16, tag="hT")
            nc.scalar.activation(hT[:], ps_h, AF.Relu, bias=b_g14, scale=1.0)

            # CNF: 3 Euler steps in T layout (x kept bf16)
            xcur = cstq
            for st in range(N_STEPS):
                nc.tensor.matmul(ps_c, W("Wc_t"), xcur[:], start=True, stop=False)
                nc.tensor.matmul(ps_c, W("Wc_b"), mdT, start=False, stop=True)
                vT = pq.tile([128, QT * D], dt.bfloat16, tag=f"vT{st}")
                nc.scalar.activation(vT[:], ps_c, AF.Tanh, bias=b_cnf4, scale=1.0)
                xn = pq.tile([128, QT * D], dt.bfloat16, tag=f"xn{st}")
                nc.vector.scalar_tensor_tensor(out=xn[:], in0=vT[:], scalar=DT_STEP,
                                               in1=xcur[:], op0=ALU.mult, op1=ALU.add)
                xcur = xn

            # func expert: cs + (0.5 + 0.5*tau) * (tanh(agg) - cs)
            d2 = pq.tile([128, QT * D], dt.bfloat16, tag="d2")
            nc.vector.tensor_tensor(out=d2[:], in0=taggT[:], in1=cstq[:],
                                    op=ALU.subtract)
            f1 = pq.tile([128, QT * D], dt.bfloat16, tag="f1")
            nc.vector.scalar_tensor_tensor(out=f1[:], in0=tauT[:], scalar=1.0,
                                           in1=d2[:], op0=ALU.add, op1=ALU.mult)
            funcT = pq.tile([128, QT * D], dt.bfloat16, tag="funcT")
            nc.vector.scalar_tensor_tensor(out=funcT[:], in0=f1[:], scalar=0.5,
                                           in1=cstq[:], op0=ALU.mult, op1=ALU.add)

            # ---- gating: softmax over 3 experts ----
            ps_g = ps_x
            nc.tensor.matmul(ps_g[0:12, 0:128], wg2k, hT[:], start=True, stop=True)
            eg = pq.tile([128, QT * D], dt.bfloat16, tag="eg")
            nc.scalar.activation(eg[0:12, :], ps_g[0:12, 0:128], AF.Exp,
                                 bias=bg2c, scale=1.0)
            ps_dn = ps_x
            nc.tensor.matmul(ps_dn[0:12, 0:128], sden, eg[0:12, :], start=True, stop=True)
            rinv = pq.tile([128, QT * D], dt.float32, tag="rinv")
            nc.vector.reciprocal(rinv[0:12, :], ps_dn[0:12, 0:128])
            gts = pq.tile([128, QT * D], dt.bfloat16, tag="gts")
            nc.vector.tensor_tensor(out=gts[0:12, :], in0=eg[0:12, :],
                                    in1=rinv[0:12, :], op=ALU.mult)

            # replicate gates over feature partitions: [(g,d), (t,c)] x3
            for e in range(3):
                nc.tensor.matmul(ps_w[:, e * QT * D:(e + 1) * QT * D],
                                 W(f"REPe{e}")[0:12, :], gts[0:12, :],
                                 start=True, stop=True)
            wrep = pq.tile([128, 3 * QT * D], dt.bfloat16, tag="wrep")
            nc.scalar.copy(wrep[:], ps_w)

            # ---- weighted combine (all bf16 2x) ----
            a0 = pq.tile([128, QT * D], dt.bfloat16, tag="a0")
            nc.vector.tensor_tensor(out=a0[:], in0=localT[:],
                                    in1=wrep[:, 0:QT * D], op=ALU.mult)
            a1 = pq.tile([128, QT * D], dt.bfloat16, tag="a1")
            nc.vector.tensor_tensor(out=a1[:], in0=funcT[:],
                                    in1=wrep[:, QT * D:2 * QT * D], op=ALU.mult)
            a2 = pq.tile([128, QT * D], dt.bfloat16, tag="a2")
            nc.vector.tensor_tensor(out=a2[:], in0=xcur[:],
                                    in1=wrep[:, 2 * QT * D:3 * QT * D], op=ALU.mult)
            s01 = pq.tile([128, QT * D], dt.bfloat16, tag="s01")
            nc.vector.tensor_tensor(out=s01[:], in0=a0[:], in1=a1[:], op=ALU.add)
            outq = pq.tile([128, QT * D], dt.float32, tag="outq")
            nc.vector.tensor_tensor(out=outq[:], in0=s01[:], in1=a2[:], op=ALU.add)

            nc.gpsimd.dma_start(a_out[:, q * QT * D:(q + 1) * QT * D], outq[:])


# ---------------------------------------------------------------------------
# host staging
# ---------------------------------------------------------------------------

def _kron4(w):
    return np.kron(np.eye(4, dtype=np.float32), w)


def stage_inputs(inputs, bs=BS, ct=CT):
    nt = bs // ct
    f32 = np.float32
    cs = np.asarray(inputs["current_state"], f32)
    nb = np.asarray(inputs["neighbor_states"], f32)
    tiers = np.asarray(inputs["tier_ids"], np.int32)

    W_local = np.asarray(inputs["W_local"], f32)
    W_msg = np.asarray(inputs["W_msg"], f32)
    W_upd = np.asarray(inputs["W_upd"], f32)
    W_cnf = np.asarray(inputs["W_cnf"], f32)
    W_g1 = np.asarray(inputs["W_g1"], f32)
    W_g2 = np.asarray(inputs["W_g2"], f32)
    b_msg = np.asarray(inputs["b_msg"], f32)
    b_local = np.asarray(inputs["b_local"], f32)
    b_upd = np.asarray(inputs["b_upd"], f32)
    b_cnf = np.asarray(inputs["b_cnf"], f32)
    b_g1 = np.asarray(inputs["b_g1"], f32)
    b_g2 = np.asarray(inputs["b_g2"], f32)

    # --- weight constants ---
    wcq = np.zeros((128, WC_COLS), f32)

    def put(name, m):
        wcq[:m.shape[0], _wslot(name):_wslot(name) + m.shape[1]] = m

    put("W4msg", _kron4(W_msg))
    put("Wl_t", _kron4(W_local[:D]))
    put("Wl_b", _kron4(W_local[D:]))
    put("Wu_t", _kron4(W_upd[:D]))
    put("Wu_b", _kron4(W_upd[D:]))
    put("Wc_t", _kron4(W_cnf[:D]))
    put("Wc_b", _kron4(W_cnf[D:]))
    put("Wg1_t", _kron4(W_g1[:D]))
    put("Wg1_b", _kron4(W_g1[D:] / K))
    put("I128", np.eye(128, dtype=f32))
    put("REP4", np.kron(np.eye(4, dtype=f32), np.ones((1, D), f32)))
    for e in range(3):
        sel = np.zeros((3, D), f32)
        sel[e, :] = 1.0
        put(f"REPe{e}", np.kron(np.eye(4, dtype=f32), sel))
    base = 128 * len(_WSLOTS)
    wcq[:, base:base + 12] = np.kron(np.eye(4, dtype=f32), W_g2)
    wcq[0:12, base + 12:base + 24] = np.kron(np.eye(4, dtype=f32),
                                             np.ones((3, 3), f32))
    wcq = wcq.astype(bf16)

    bcq = np.zeros((128, BC_COLS), f32)
    bcq[:, 0] = np.tile(b_msg, 4)
    bcq[:, 1] = np.tile(b_local, 4)
    bcq[:, 2] = 0.5 * np.tile(b_upd, 4)
    bcq[:, 3] = np.tile(b_cnf, 4)
    bcq[:, 4] = np.tile(b_g1, 4)
    bcq[0:12, 5] = np.tile(b_g2, 4)

    in_maps = []
    for c in range(N_CORES):
        rs = slice(c * bs, (c + 1) * bs)
        nb_c = nb[rs]
        cs_c = cs[rs]
        tr_c = tiers[rs]

        m0 = (tr_c == 0)
        m1 = (tr_c == 1)
        m2 = (tr_c == 2)
        cnt0 = np.maximum(m0.sum(-1), 1).astype(f32)
        cnt1 = np.maximum(m1.sum(-1), 1).astype(f32)
        cnt2 = np.maximum(m2.sum(-1), 1).astype(f32)

        # T layout, tier-1 premasked: nbt[(g,d), t, (c,k)]
        nbm = nb_c * m1[:, :, None].astype(f32)
        arr = nbm.reshape(nt, 4, 32, K, D).transpose(1, 4, 0, 2, 3)
        nbt = np.ascontiguousarray(arr).reshape(128, nt * FR).astype(bf16)

        # natural (d,k): nbn[cell, d*K + k]
        nbn = nb_c.transpose(0, 2, 1).reshape(bs, FR).astype(bf16)

        aux = np.empty((bs, 2 * K), f16)
        aux[:, 0:K] = (m0 / cnt0[:, None]).astype(f16)
        aux[:, K:2 * K] = (m2 / cnt2[:, None]).astype(f16)

        cst = cs_c.reshape(nt, 4, 32, D).transpose(1, 3, 0, 2) \
            .reshape(128, nt * D).astype(bf16)

        scl = (1.0 / cnt1).reshape(nt, 4, 32).transpose(1, 0, 2) \
            .reshape(4, nt * D).astype(bf16)

        in_maps.append({
            "nbt": nbt, "nbn": nbn, "aux": aux, "cst": cst, "scl": scl,
            "wc": wcq, "bc": bcq,
        })
    return in_maps


def unstage_output(outt, bs=BS, ct=CT):
    """outt [128, nt*D] T layout -> [bs, D] natural."""
    nt = bs // ct
    return np.ascontiguousarray(
        outt.reshape(4, D, nt, 32).transpose(2, 0, 3, 1).reshape(bs, D))


_PROGRAM_CACHE = {}


def kernel(**inputs):
    from concourse.bass_utils import run_bass_kernel_spmd

    key = (BS, CT)
    if key not in _PROGRAM_CACHE:
        _PROGRAM_CACHE[key] = build_program(BS, CT)
    nc = _PROGRAM_CACHE[key]

    in_maps = stage_inputs(inputs, BS, CT)
    res = run_bass_kernel_spmd(nc, in_maps, core_ids=list(range(N_CORES)))
    out = np.concatenate(
        [unstage_output(r["outt"].astype(np.float32)) for r in res.results],
        axis=0)
    return out.astype(np.float32)


# revision 8
# speedup vs baseline: 1.6587x; 1.0644x over previous
"""Trainium2 Bass kernel for nn_MoEConnectionProcessor.

Self-contained: stages/shards the full inputs on host (numpy), runs an SPMD
Bass/Tile kernel on 8 NeuronCores, gathers the full output.

Reference math (per cell, K=26 neighbors, D=32):
  masks by tier (0=local,1=functional,2=distant); masked neighbor means;
  local expert  = tanh([cs, loc_mean] @ W_local + b_local)
  func expert   = (1-z)*cs + z*tanh(agg),  z = sigmoid([cs, agg] @ W_upd + b_upd)
                  agg = masked_mean_k tanh(nb @ W_msg + b_msg)
  dist expert   = 3-step Euler: x += (1/3) tanh([x, agg_d] @ W_cnf + b_cnf)
  gates         = softmax([cs, mean_nb] @ W_g1 + b_g1 -> relu -> @ W_g2 + b_g2)
  out           = sum_t gate_t * expert_t

Device strategy (per 128-cell tile, Q=4 tiles batched for the small ops):
  - nb staged twice from host: T layout [(g,d), (c,k)] PRE-MASKED by the
    tier-1 mask (so tanh gives exact zeros for non-functional neighbors and
    the functional aggregate is a plain k-reduce), and natural [c, (d,k)]
    raw with k innermost (so the masked multiplies run in DVE 2x mode with
    the per-(cell,k) weights broadcast along d as an outer dim).
  - tier-0/tier-2 means: one fused 2x multiply by pre-divided weights
    (m_t/cnt_t, fp16) + one fused 1x k-reduce.
  - S0 (gating mean): PE identity-accumulation into PSUM (26 matmuls).
  - sigmoid via tanh identity, relu/exp/copy on ACT: every activation is
    served by the "exp_and_others" table -> zero ACT table reloads.
  - experts/gating/combine all in T layout on [128, 4*32] batched operands;
    per-cell gates/scales replicated across partitions with tiny PE matmuls;
    output staged in T layout, un-transposed on host.
"""

import numpy as np
import ml_dtypes
from contextlib import ExitStack

import concourse.bass as bass
import concourse.bacc as bacc
import concourse.tile as tile
import concourse.mybir as mybir

B, K, D, NH = 262144, 26, 32, 32
N_CORES = 8
BS = B // N_CORES   # 32768 cells per core
CT = 128            # cells per tile
QT = 4              # tiles per batch-group
N_STEPS = 3
DT_STEP = 1.0 / N_STEPS

dt = mybir.dt
bf16 = ml_dtypes.bfloat16
f16 = np.float16
AF = mybir.ActivationFunctionType
ALU = mybir.AluOpType
AXX = mybir.AxisListType

FR = K * D  # 832
TW = 2 * FR + 2 * K  # 1716: [nbt 832 | nbn 832 | aux 52] packed per tile

# weight-constant dram tensor [128, WC_COLS] bf16 layout
_WSLOTS = ["W4msg", "Wl_t", "Wl_b", "Wu_t", "Wu_b", "Wc_t", "Wc_b",
           "Wg1_t", "Wg1_b", "I128", "REP4", "REPe0", "REPe1", "REPe2"]
_WEXTRA = 24  # WG2K [128,12] + SDEN [12,12 in a 12-col slot]
WC_COLS = 128 * len(_WSLOTS) + _WEXTRA
BC_COLS = 8


def _wslot(name):
    return 128 * _WSLOTS.index(name)


def build_program(bs=BS, ct=CT):
    nt = bs // ct
    nq = nt // QT
    nc = bacc.Bacc("TRN2", target_bir_lowering=False, debug=False,
                   num_devices=N_CORES)

    a_nbig = nc.dram_tensor("nbig", [128, nt * TW], dt.bfloat16, kind="ExternalInput").ap()
    a_cstm = nc.dram_tensor("cstm", [128, nt * 64], dt.bfloat16, kind="ExternalInput").ap()
    a_wc = nc.dram_tensor("wc", [128, WC_COLS], dt.bfloat16, kind="ExternalInput").ap()
    a_bc = nc.dram_tensor("bc", [128, BC_COLS], dt.float32, kind="ExternalInput").ap()
    a_out = nc.dram_tensor("outt", [128, nt * D], dt.float32, kind="ExternalOutput").ap()

    with tile.TileContext(nc) as tc:
        _body(tc, a_nbig, a_cstm, a_wc, a_bc, a_out, bs, ct, nt, nq)
    nc.compile()
    return nc


def _body(tc, a_nbig, a_cstm, a_wc, a_bc, a_out, bs, ct, nt, nq):
    nc = tc.nc

    with ExitStack() as ctx:
        ctx.enter_context(nc.allow_low_precision("reduce output downcast; fp32 internal accum"))
        cpool = ctx.enter_context(tc.tile_pool(name="const", bufs=1))
        pin = ctx.enter_context(tc.tile_pool(name="in", bufs=3))
        pmid = ctx.enter_context(tc.tile_pool(name="mid", bufs=2))
        pq = ctx.enter_context(tc.tile_pool(name="q", bufs=2))
        psm = ctx.enter_context(tc.tile_pool(name="psm", bufs=2, space="PSUM"))
        psq = ctx.enter_context(tc.tile_pool(name="psq", bufs=2, space="PSUM"))
        psb = ctx.enter_context(tc.tile_pool(name="psb", bufs=1, space="PSUM"))

        wc = cpool.tile([128, WC_COLS], dt.bfloat16, tag="wc")
        nc.sync.dma_start(wc[:], a_wc)
        bc = cpool.tile([128, BC_COLS], dt.float32, tag="bc")
        nc.sync.dma_start(bc[:], a_bc)

        def W(name):
            return wc[:, _wslot(name): _wslot(name) + 128]

        wg2k = wc[:, 128 * len(_WSLOTS): 128 * len(_WSLOTS) + 12]
        sden = wc[0:12, 128 * len(_WSLOTS) + 12: 128 * len(_WSLOTS) + 24]
        b_msg4 = bc[:, 0:1]
        b_loc4 = bc[:, 1:2]
        hb_upd4 = bc[:, 2:3]   # 0.5 * b_upd (for the tanh-sigmoid identity)
        b_cnf4 = bc[:, 3:4]
        b_g14 = bc[:, 4:5]
        bg2c = bc[0:12, 5:6]   # b_g2 at (g,e) partitions

        for q in range(nq):
            # ---- per-Q staging tiles (filled by phase A) ----
            ML = pq.tile([128, QT * 2 * D], dt.bfloat16, tag="ML")   # [c,(t,s,d)]
            SGT = pq.tile([128, QT * D], dt.bfloat16, tag="SGT")     # [(g,j),(t,c)]
            ps_s0 = psq.tile([128, QT * D], dt.float32, tag="ps_s0")  # [c,(t,d)]

            # ================= phase A: per tile =================
            for s in range(QT):
                t = q * QT + s

                big = pin.tile([128, TW], dt.bfloat16, tag="big")
                nc.gpsimd.dma_start(big[:], a_nbig[:, t * TW:(t + 1) * TW])
                nbt = big[:, 0:FR]
                nbn = big[:, FR:2 * FR]
                aux = big[:, 2 * FR:TW].bitcast(dt.float16)

                nbn3 = nbn.rearrange("p (d k) -> p d k", d=D)

                # tier-0 product on DVE, tier-2 product on GpSimd (both 2x-ish)
                prod = pmid.tile([128, 2 * FR], dt.bfloat16, tag="prod")
                pview = prod[:].rearrange("p (s d k) -> p s d k", s=2, d=D)
                aview = aux.rearrange("p (s k) -> p s k", s=2)
                nc.vector.tensor_tensor(
                    out=pview[:, 0],
                    in0=nbn3,
                    in1=aview[:, 0].unsqueeze(1).to_broadcast((128, D, K)),
                    op=ALU.mult)
                nc.gpsimd.tensor_tensor(
                    out=pview[:, 1],
                    in0=nbn3,
                    in1=aview[:, 1].unsqueeze(1).to_broadcast((128, D, K)),
                    op=ALU.mult)

                # halve k by one 2x pair-add, then 1x-reduce 13 -> means
                padd = pmid.tile([128, 2 * D * 13], dt.bfloat16, tag="padd")
                pav = padd[:].rearrange("p (s d k) -> p s d k", s=2, d=D)
                nc.vector.tensor_tensor(out=pav, in0=pview[:, :, :, 0:13],
                                        in1=pview[:, :, :, 13:26], op=ALU.add)
                mlv = ML[:].rearrange("p (t s d) -> p t s d", t=QT, s=2)
                nc.vector.tensor_reduce(out=mlv[:, s], in_=pav,
                                        axis=AXX.X, op=ALU.add)

                # messages: premsgs = kron4(W_msg) @ nbt  (nbt pre-masked m1)
                ps_m0 = psm.tile([128, 416], dt.float32, tag="psm0")
                ps_m1 = psm.tile([128, 416], dt.float32, tag="psm1")
                nc.tensor.matmul(ps_m0[:], W("W4msg"), nbt[:, 0:416],
                                 start=True, stop=True)
                nc.tensor.matmul(ps_m1[:], W("W4msg"), nbt[:, 416:832],
                                 start=True, stop=True)
                mt = pmid.tile([128, FR], dt.bfloat16, tag="msgsT")
                nc.scalar.activation(mt[:, 0:416], ps_m0[:], AF.Tanh,
                                     bias=b_msg4, scale=1.0)
                nc.scalar.activation(mt[:, 416:832], ps_m1[:], AF.Tanh,
                                     bias=b_msg4, scale=1.0)

                # functional aggregate: 2x pair-add + 1x k-reduce
                mt3 = mt[:].rearrange("p (c k) -> p c k", k=K)
                spad = pmid.tile([128, D * 13], dt.bfloat16, tag="spad")
                spv = spad[:].rearrange("p (c k) -> p c k", k=13)
                nc.vector.tensor_tensor(out=spv, in0=mt3[:, :, 0:13],
                                        in1=mt3[:, :, 13:26], op=ALU.add)
                nc.vector.tensor_reduce(out=SGT[:, s * D:(s + 1) * D],
                                        in_=spv, axis=AXX.X, op=ALU.add)

                # S0 = sum_k nb  (PE identity accumulation)
                for k in range(K):
                    nc.tensor.matmul(ps_s0[:, s * D:(s + 1) * D], W("I128"),
                                     nbn3[:, :, k],
                                     start=(k == 0), stop=(k == K - 1))

            # ================= phase B: per Q-group =================
            cm = pin.tile([128, QT * 64], dt.bfloat16, tag="cm")
            nc.gpsimd.dma_start(cm[:], a_cstm[:, q * QT * 64:(q + 1) * QT * 64])
            cmv = cm[:].rearrange("p (t x) -> p t x", x=64)
            cstq = cmv[:, :, 0:D]          # [128, (t, c)] current-state T
            sclq4 = cm[0:4, :].rearrange("p (t x) -> p t x", x=64)[:, :, D:2 * D]

            # T-transpose of the tier means (32x32 block transpose)
            MLT = pq.tile([128, QT * 2 * D], dt.bfloat16, tag="MLT")
            nc.vector.transpose(MLT[:], ML[:])
            mltv = MLT[:].rearrange("p (t s c) -> p t s c", t=QT, s=2)
            mlT = mltv[:, :, 0]   # [(g,d), (t,c)] tier-0 means
            mdT = mltv[:, :, 1]   # tier-2 means

            # S0 -> bf16 -> T layout (gating mean; 1/K folded into Wg1_b)
            s0b = pq.tile([128, QT * D], dt.bfloat16, tag="s0b")
            nc.scalar.copy(s0b[:], ps_s0[:])
            mnT = pq.tile([128, QT * D], dt.bfloat16, tag="mnT")
            nc.vector.transpose(mnT[:], s0b[:])

            # two shared PSUM bank tiles for all phase-B matmul outputs
            PB1 = psb.tile([128, 512], dt.float32, tag="PB1")
            PB2 = psb.tile([128, 512], dt.float32, tag="PB2")
            ps_l = PB1[:, 0:128]
            ps_z = PB1[:, 128:256]
            ps_h = PB1[:, 256:384]
            ps_c = PB1[:, 384:512]
            ps_w = PB2[:, 0:384]
            ps_x = PB2[:, 384:512]   # reused: scl-rep, gating logits, denom

            # replicate 1/cnt1 across partitions; scale + tanh the aggregate
            ps_scl = ps_x
            nc.tensor.matmul(ps_scl, W("REP4")[0:4, :], sclq4,
                             start=True, stop=True)
            aggTs = pq.tile([128, QT * D], dt.bfloat16, tag="aggTs")
            nc.vector.tensor_tensor(out=aggTs[:], in0=SGT[:], in1=ps_scl,
                                    op=ALU.mult)
            taggT = pq.tile([128, QT * D], dt.bfloat16, tag="taggT")
            nc.scalar.activation(taggT[:], aggTs[:], AF.Tanh)

            # ---- experts (batched matmuls over [128, QT*32]) ----
            nc.tensor.matmul(ps_l, W("Wl_t"), cstq, start=True, stop=False)
            nc.tensor.matmul(ps_l, W("Wl_b"), mlT, start=False, stop=True)
            localT = pq.tile([128, QT * D], dt.bfloat16, tag="localT")
            nc.scalar.activation(localT[:], ps_l, AF.Tanh, bias=b_loc4, scale=1.0)

            nc.tensor.matmul(ps_z, W("Wu_t"), cstq, start=True, stop=False)
            nc.tensor.matmul(ps_z, W("Wu_b"), aggTs[:], start=False, stop=True)
            tauT = pq.tile([128, QT * D], dt.bfloat16, tag="tauT")
            nc.scalar.activation(tauT[:], ps_z, AF.Tanh, bias=hb_upd4, scale=0.5)

            nc.tensor.matmul(ps_h, W("Wg1_t"), cstq, start=True, stop=False)
            nc.tensor.matmul(ps_h, W("Wg1_b"), mnT[:], start=False, stop=True)
            hT = pq.tile([128, QT * D], dt.bfloat16, tag="hT")
            nc.scalar.activation(hT[:], ps_h, AF.Relu, bias=b_g14, scale=1.0)

            # CNF: 3 Euler steps in T layout (x kept bf16)
            xcur = cstq
            for st in range(N_STEPS):
                xm = xcur if xcur is cstq else xcur[:]
                nc.tensor.matmul(ps_c, W("Wc_t"), xm, start=True, stop=False)
                nc.tensor.matmul(ps_c, W("Wc_b"), mdT, start=False, stop=True)
                vT = pq.tile([128, QT * D], dt.bfloat16, tag=f"vT{st}")
                nc.scalar.activation(vT[:], ps_c, AF.Tanh, bias=b_cnf4, scale=1.0)
                xn = pq.tile([128, QT * D], dt.bfloat16, tag=f"xn{st}")
                xi = xcur if xcur is cstq else xcur[:]
                nc.vector.scalar_tensor_tensor(out=xn[:], in0=vT[:], scalar=DT_STEP,
                                               in1=xi, op0=ALU.mult, op1=ALU.add)
                xcur = xn

            # func expert: cs + (0.5 + 0.5*tau) * (tanh(agg) - cs)
            d2 = pq.tile([128, QT * D], dt.bfloat16, tag="d2")
            nc.vector.tensor_tensor(out=d2[:], in0=taggT[:], in1=cstq,
                                    op=ALU.subtract)
            f1 = pq.tile([128, QT * D], dt.bfloat16, tag="f1")
            nc.vector.scalar_tensor_tensor(out=f1[:], in0=tauT[:], scalar=1.0,
                                           in1=d2[:], op0=ALU.add, op1=ALU.mult)
            funcT = pq.tile([128, QT * D], dt.bfloat16, tag="funcT")
            nc.vector.scalar_tensor_tensor(out=funcT[:], in0=f1[:], scalar=0.5,
                                           in1=cstq, op0=ALU.mult, op1=ALU.add)

            # ---- gating: softmax over 3 experts ----
            ps_g = ps_x
            nc.tensor.matmul(ps_g[0:12, 0:128], wg2k, hT[:], start=True, stop=True)
            eg = pq.tile([128, QT * D], dt.bfloat16, tag="eg")
            nc.scalar.activation(eg[0:12, :], ps_g[0:12, 0:128], AF.Exp,
                                 bias=bg2c, scale=1.0)
            ps_dn = ps_x
            nc.tensor.matmul(ps_dn[0:12, 0:128], sden, eg[0:12, :], start=True, stop=True)
            rinv = pq.tile([128, QT * D], dt.float32, tag="rinv")
            nc.vector.reciprocal_approx_fast(rinv[0:12, :], ps_dn[0:12, 0:128])
            gts = pq.tile([128, QT * D], dt.bfloat16, tag="gts")
            nc.vector.tensor_tensor(out=gts[0:12, :], in0=eg[0:12, :],
                                    in1=rinv[0:12, :], op=ALU.mult)

            # replicate gates over feature partitions: [(g,d), (t,c)] x3
            for e in range(3):
                nc.tensor.matmul(ps_w[:, e * QT * D:(e + 1) * QT * D],
                                 W(f"REPe{e}")[0:12, :], gts[0:12, :],
                                 start=True, stop=True)
            wrep = pq.tile([128, 3 * QT * D], dt.bfloat16, tag="wrep")
            nc.scalar.copy(wrep[:], ps_w)

            # ---- weighted combine (all bf16 2x) ----
            a0 = pq.tile([128, QT * D], dt.bfloat16, tag="a0")
            nc.vector.tensor_tensor(out=a0[:], in0=localT[:],
                                    in1=wrep[:, 0:QT * D], op=ALU.mult)
            a1 = pq.tile([128, QT * D], dt.bfloat16, tag="a1")
            nc.vector.tensor_tensor(out=a1[:], in0=funcT[:],
                                    in1=wrep[:, QT * D:2 * QT * D], op=ALU.mult)
            a2 = pq.tile([128, QT * D], dt.bfloat16, tag="a2")
            nc.vector.tensor_tensor(out=a2[:], in0=xcur[:],
                                    in1=wrep[:, 2 * QT * D:3 * QT * D], op=ALU.mult)
            s01 = pq.tile([128, QT * D], dt.bfloat16, tag="s01")
            nc.vector.tensor_tensor(out=s01[:], in0=a0[:], in1=a1[:], op=ALU.add)
            outq = pq.tile([128, QT * D], dt.float32, tag="outq")
            nc.vector.tensor_tensor(out=outq[:], in0=s01[:], in1=a2[:], op=ALU.add)

            nc.gpsimd.dma_start(a_out[:, q * QT * D:(q + 1) * QT * D], outq[:])


# ---------------------------------------------------------------------------
# host staging
# ---------------------------------------------------------------------------

def _kron4(w):
    return np.kron(np.eye(4, dtype=np.float32), w)


def stage_inputs(inputs, bs=BS, ct=CT):
    nt = bs // ct
    f32 = np.float32
    cs = np.asarray(inputs["current_state"], f32)
    nb = np.asarray(inputs["neighbor_states"], f32)
    tiers = np.asarray(inputs["tier_ids"], np.int32)

    W_local = np.asarray(inputs["W_local"], f32)
    W_msg = np.asarray(inputs["W_msg"], f32)
    W_upd = np.asarray(inputs["W_upd"], f32)
    W_cnf = np.asarray(inputs["W_cnf"], f32)
    W_g1 = np.asarray(inputs["W_g1"], f32)
    W_g2 = np.asarray(inputs["W_g2"], f32)
    b_msg = np.asarray(inputs["b_msg"], f32)
    b_local = np.asarray(inputs["b_local"], f32)
    b_upd = np.asarray(inputs["b_upd"], f32)
    b_cnf = np.asarray(inputs["b_cnf"], f32)
    b_g1 = np.asarray(inputs["b_g1"], f32)
    b_g2 = np.asarray(inputs["b_g2"], f32)

    # --- weight constants ---
    wcq = np.zeros((128, WC_COLS), f32)

    def put(name, m):
        wcq[:m.shape[0], _wslot(name):_wslot(name) + m.shape[1]] = m

    put("W4msg", _kron4(W_msg))
    put("Wl_t", _kron4(W_local[:D]))
    put("Wl_b", _kron4(W_local[D:]))
    put("Wu_t", _kron4(W_upd[:D]))
    put("Wu_b", _kron4(W_upd[D:]))
    put("Wc_t", _kron4(W_cnf[:D]))
    put("Wc_b", _kron4(W_cnf[D:]))
    put("Wg1_t", _kron4(W_g1[:D]))
    put("Wg1_b", _kron4(W_g1[D:] / K))
    put("I128", np.eye(128, dtype=f32))
    put("REP4", np.kron(np.eye(4, dtype=f32), np.ones((1, D), f32)))
    for e in range(3):
        sel = np.zeros((3, D), f32)
        sel[e, :] = 1.0
        put(f"REPe{e}", np.kron(np.eye(4, dtype=f32), sel))
    base = 128 * len(_WSLOTS)
    wcq[:, base:base + 12] = np.kron(np.eye(4, dtype=f32), W_g2)
    wcq[0:12, base + 12:base + 24] = np.kron(np.eye(4, dtype=f32),
                                             np.ones((3, 3), f32))
    wcq = wcq.astype(bf16)

    bcq = np.zeros((128, BC_COLS), f32)
    bcq[:, 0] = np.tile(b_msg, 4)
    bcq[:, 1] = np.tile(b_local, 4)
    bcq[:, 2] = 0.5 * np.tile(b_upd, 4)
    bcq[:, 3] = np.tile(b_cnf, 4)
    bcq[:, 4] = np.tile(b_g1, 4)
    bcq[0:12, 5] = np.tile(b_g2, 4)

    in_maps = []
    for c in range(N_CORES):
        rs = slice(c * bs, (c + 1) * bs)
        nb_c = nb[rs]
        cs_c = cs[rs]
        tr_c = tiers[rs]

        m0 = (tr_c == 0)
        m1 = (tr_c == 1)
        m2 = (tr_c == 2)
        cnt0 = np.maximum(m0.sum(-1), 1).astype(f32)
        cnt1 = np.maximum(m1.sum(-1), 1).astype(f32)
        cnt2 = np.maximum(m2.sum(-1), 1).astype(f32)

        # T layout, tier-1 premasked: nbt[(g,d), t, (c,k)]
        nbm = nb_c * m1[:, :, None].astype(f32)
        arr = nbm.reshape(nt, 4, 32, K, D).transpose(1, 4, 0, 2, 3)
        nbt = np.ascontiguousarray(arr).reshape(128, nt, FR).astype(bf16)

        # natural (d,k): nbn[cell, d*K + k]
        nbn = nb_c.transpose(0, 2, 1).reshape(nt, 128, FR).astype(bf16)

        aux = np.empty((bs, 2 * K), f16)
        aux[:, 0:K] = (m0 / cnt0[:, None]).astype(f16)
        aux[:, K:2 * K] = (m2 / cnt2[:, None]).astype(f16)

        nbig = np.empty((128, nt, TW), np.uint16)
        nbig[:, :, 0:FR] = nbt.view(np.uint16)
        nbig[:, :, FR:2 * FR] = nbn.view(np.uint16).transpose(1, 0, 2)
        nbig[:, :, 2 * FR:TW] = aux.view(np.uint16) \
            .reshape(nt, 128, 2 * K).transpose(1, 0, 2)
        nbig = nbig.reshape(128, nt * TW).view(bf16)

        cst = cs_c.reshape(nt, 4, 32, D).transpose(1, 3, 0, 2) \
            .reshape(128, nt, D).astype(bf16)
        scl = (1.0 / cnt1).reshape(nt, 4, 32).transpose(1, 0, 2) \
            .reshape(4, nt, D).astype(bf16)
        cstm = np.zeros((128, nt, 64), bf16)
        cstm[:, :, 0:D] = cst
        cstm[0:4, :, D:2 * D] = scl
        cstm = cstm.reshape(128, nt * 64)

        in_maps.append({
            "nbig": nbig, "cstm": cstm, "wc": wcq, "bc": bcq,
        })
    return in_maps


def unstage_output(outt, bs=BS, ct=CT):
    """outt [128, nt*D] T layout -> [bs, D] natural."""
    nt = bs // ct
    return np.ascontiguousarray(
        outt.reshape(4, D, nt, 32).transpose(2, 0, 3, 1).reshape(bs, D))


_PROGRAM_CACHE = {}


def kernel(**inputs):
    from concourse.bass_utils import run_bass_kernel_spmd

    key = (BS, CT)
    if key not in _PROGRAM_CACHE:
        _PROGRAM_CACHE[key] = build_program(BS, CT)
    nc = _PROGRAM_CACHE[key]

    in_maps = stage_inputs(inputs, BS, CT)
    res = run_bass_kernel_spmd(nc, in_maps, core_ids=list(range(N_CORES)))
    out = np.concatenate(
        [unstage_output(r["outt"].astype(np.float32)) for r in res.results],
        axis=0)
    return out.astype(np.float32)
